# revision 21
# baseline (speedup 1.0000x reference)
"""Additive LoRA adapter (MoE-routed) forward — Trainium2, 8 NeuronCores.

Data-parallel over tokens: each core gets n/8 tokens, weights replicated.
Forward only => no collectives.

Per-core compute (feature-major / "transposed activations" layout):
  - base:   out.T[o,t] += sum_k Wt[k,o-block].T @ xbf[k,t]     (bf16 matmul)
  - router: h[hid,t] = silu(rw1t.T @ xf32 + rb1)               (bf16 matmul)
            logits[t,e] = (h_aug[.,t-block]).T @ rw2_aug       (fp32, bias+gates
            via the appended ones-row of h_aug)
  - top2 + softmax on VectorE via exp / two maxes / masks
  - coeff.T via PE transpose, expanded to (e,r)-rows via a scaled 0/1
    expand matmul (entries ALPHA/128 fold the fp8 scales back out)
  - xa.T[(e,r),t] = At8.T @ x8 as fp8e4 DoubleRow matmuls (2x PE rate);
    A is pre-scaled by 32 so fp8e4 stays out of the subnormal range
  - wxa8 = xa_s * (coeff/128) quantized to fp8e4 on DVE
  - delta accumulated into the same PSUM groups as base via fp8 DoubleRow
    against B pre-scaled by 4 (32 * 4 / 128 = 1 => unscaled delta).
    Output chunks are processed in PAIRS so the bf16<->fp8 PE mode switch
    (~0.25us each way) is paid once per pair, not once per chunk.
  - router h stays bf16: fp8 logit noise flips ~7% of top-2 picks, which
    costs ~1.5e-2 rel err; bf16 keeps it at ~5e-3.
Host pre-transposes/quantizes so every matmul operand is a natural
(contraction-on-partitions) SBUF load; weights/x are packed so every DMA
moves >=2KB contiguous per partition (small-line DMAs starve startup).
Output is produced transposed in bf16 and un-transposed on the host.
"""
import sys

sys.path.insert(0, "/opt/trn_rl_repo")

import numpy as np
import ml_dtypes

from concourse import bacc, tile, mybir
from concourse.bass_utils import run_bass_kernel_spmd

N_CORES = 8
D = 2048          # d_in == d_out
E = 16            # populated experts
R = 16            # lora rank
ER = E * R        # 256
HID = 64          # router hidden
P = 128           # partitions
KC = D // P       # 16 contraction chunks
KCP = KC // 2     # 8 contraction chunk-pairs (fp8 DoubleRow)
OC = D // P       # 16 output chunks
TT = 512          # token tile
TH = TT // 2      # fp8 DoubleRow moving-token chunk (256)
ALPHA = 1.0
WARM_MM = 58      # PE warm-up matmuls covering DMA spin-up

# fp8 scaling: A8 = 32*A, B8 = 4*B, expand entries ALPHA/128 => delta unscaled
SC_A = 32.0
SC_B = 4.0
SC_E = ALPHA / (SC_A * SC_B)

F32 = mybir.dt.float32
BF16 = mybir.dt.bfloat16
F8 = mybir.dt.float8e4
AF = mybir.ActivationFunctionType
ALU = mybir.AluOpType
DR = mybir.MatmulPerfMode.DoubleRow
NEG_BIG = -1.0e30


def _build(n_core: int):
    NT = n_core // TT
    nc = bacc.Bacc("TRN2", target_bir_lowering=False, debug=False,
                   num_devices=N_CORES)

    # x bf16 blocked [NT, P, KC, TT] -> 4KB-line DMAs (k-quarters)
    xbf_d = nc.dram_tensor("xbf", [NT, P, KC, TT], BF16,
                           kind="ExternalInput").ap()
    # x fp8 k-paired [NT, P, KCP, 2, TT] -> DoubleRow moving operand
    x8_d = nc.dram_tensor("x8", [NT, P, KCP, 2, TT], F8,
                          kind="ExternalInput").ap()
    # W.T blocked i-major: [OC, 128i, KC, 128o] -> contiguous 4KB per
    # partition per oc-slab
    wt_d = nc.dram_tensor("wt", [OC, P, KC, P], BF16, kind="ExternalInput").ap()
    # A.T fp8 k-paired [P, KCP, 2, ER] (scaled by SC_A) -> one 4KB-line DMA
    at8_d = nc.dram_tensor("at8", [P, KCP, 2, ER], F8,
                           kind="ExternalInput").ap()
    # B rows fp8 er-paired [P, 2, D] (scaled by SC_B) -> one 4KB-line DMA
    bf8_d = nc.dram_tensor("bf8", [P, 2, D], F8, kind="ExternalInput").ap()
    # router W1.T packed [P, KC, HID] -> one 2KB-line DMA
    rw1t_d = nc.dram_tensor("rw1t", [P, KC, HID], BF16,
                            kind="ExternalInput").ap()
    rb1_d = nc.dram_tensor("rb1", [HID, 1], F32, kind="ExternalInput").ap()
    rw2a_d = nc.dram_tensor("rw2a", [HID + 1, E], F32, kind="ExternalInput").ap()
    bias_d = nc.dram_tensor("biaspp", [P, OC], F32, kind="ExternalInput").ap()
    # per-s-chunk expand blocks [64se, 4s, 2half, 128j] (entries ALPHA/128)
    ex4_d = nc.dram_tensor("ex4", [4 * E, 4, 2, P], BF16,
                           kind="ExternalInput").ap()
    ident_d = nc.dram_tensor("ident", [P, P], BF16, kind="ExternalInput").ap()
    outT_d = nc.dram_tensor("outT", [D, n_core], BF16,
                            kind="ExternalOutput").ap()

    with tile.TileContext(nc) as tc:
        with (
            tc.tile_pool(name="const", bufs=1) as constp,
            tc.tile_pool(name="wres", bufs=1) as wres,
            tc.tile_pool(name="xb", bufs=3) as xbp,
            tc.tile_pool(name="x8p", bufs=3) as x8p,
            tc.tile_pool(name="hp", bufs=2) as hp,
            tc.tile_pool(name="small", bufs=4) as smallp,
            tc.tile_pool(name="cf", bufs=2) as cfp,
            tc.tile_pool(name="outp", bufs=4) as outp,
            tc.tile_pool(name="ps_out", bufs=3, space="PSUM") as ps_out,
            tc.tile_pool(name="ps_xa", bufs=1, space="PSUM") as ps_xa,
            tc.tile_pool(name="ps_h", bufs=1, space="PSUM") as ps_h,
            tc.tile_pool(name="ps_sm", bufs=1, space="PSUM") as ps_sm,
        ):
            # ---- startup-critical loads first: router W1, A, x tile 0 ----
            rw1t_sb = constp.tile([P, KC, HID], BF16)
            nc.sync.dma_start(out=rw1t_sb[:], in_=rw1t_d[:])
            at8_sb = wres.tile([P, KCP, 2, ER], F8)
            nc.sync.dma_start(out=at8_sb[:], in_=at8_d[:])
            rb1_sb = constp.tile([HID, 1], F32)
            nc.sync.dma_start(out=rb1_sb[:], in_=rb1_d[:])
            rw2a_sb = constp.tile([HID + 1, E], F32)
            nc.sync.dma_start(out=rw2a_sb[:], in_=rw2a_d[:])

            def load_x_tile(tt):
                xb_sb = xbp.tile([P, KC, TT], BF16)
                for q in range(4):
                    nc.sync.dma_start(out=xb_sb[:, 4 * q:4 * q + 4, :],
                                      in_=xbf_d[tt, :, 4 * q:4 * q + 4, :])
                x8_sb = x8p.tile([P, KCP, 2, TT], F8)
                for hseg in range(2):
                    nc.sync.dma_start(
                        out=x8_sb[:, 4 * hseg:4 * hseg + 4, :, :],
                        in_=x8_d[tt, :, 4 * hseg:4 * hseg + 4, :, :])
                return xb_sb, x8_sb

            x_tile0 = load_x_tile(0)
            ex4_sb = constp.tile([4 * E, 4, 2, P], BF16)
            nc.sync.dma_start(out=ex4_sb[:], in_=ex4_d[:])
            ident_sb = constp.tile([P, P], BF16)
            nc.sync.dma_start(out=ident_sb[:], in_=ident_d[:])

            # ---- resident weights; tile-1 x hoisted ahead of W so the
            # tile boundary never starves ----
            x_tile1 = load_x_tile(1) if NT > 1 else None
            bias_sb = constp.tile([P, OC], F32)
            nc.sync.dma_start(out=bias_sb[:], in_=bias_d[:])
            bf8_sb = wres.tile([P, 2, D], F8)
            nc.sync.dma_start(out=bf8_sb[:], in_=bf8_d[:])
            x_tile2 = None
            wt_sb = wres.tile([P, OC, KC, P], BF16)
            for oc in range(OC):
                nc.sync.dma_start(
                    out=wt_sb[:, oc, :, :],
                    in_=wt_d[oc, :, :, :])
                if oc == 6 and NT > 2:
                    # slot tile-2's x into the W stream: lands before the
                    # pipeline needs it, without delaying early W chunks
                    x_tile2 = load_x_tile(2)

            def front(tt):
                # router + xa + top2 + wxa: everything that does NOT need W.
                # Emitted one tile ahead of back() so the PE always has
                # W-independent work while wt/xb stream in.
                if tt == 0:
                    xb_sb, x8_sb = x_tile0
                elif tt == 1:
                    xb_sb, x8_sb = x_tile1
                elif tt == 2:
                    xb_sb, x8_sb = x_tile2
                else:
                    xb_sb, x8_sb = load_x_tile(tt)

                # ---- router hidden: h = silu(rw1t.T @ x + rb1) (bf16 mm) ----
                h_ps = ps_h.tile([HID, TT], F32)
                if tt == 0:
                    # dummy matmuls on a memset scratch tile (no DMA deps):
                    # fill the DMA spin-up so the PE is busy and HAM
                    # stays at 2.4GHz when real work arrives
                    scr_sb = constp.tile([P, TT], BF16)
                    nc.vector.memset(scr_sb[:], 1.0)
                    warm_ps = ps_out.tile([P, TT], F32, tag="out")
                    for _ in range(WARM_MM):
                        nc.tensor.matmul(warm_ps[:], lhsT=scr_sb[:, 0:P],
                                         rhs=scr_sb[:], start=True,
                                         stop=True)
                for k in range(KC):
                    nc.tensor.matmul(h_ps[:], lhsT=rw1t_sb[:, k, :],
                                     rhs=xb_sb[:, k, :],
                                     start=(k == 0), stop=(k == KC - 1))
                h_sb = hp.tile([HID + 1, TT], F32)
                nc.vector.memset(h_sb[HID:HID + 1, :], 1.0)

                # ---- silu + logits per 128-token sub-chunk (fp32 mm) ----
                lg_ps = ps_sm.tile([P, TT // P, E], F32, tag="lgct")
                for s in range(TT // P):
                    nc.scalar.activation(h_sb[0:HID, s * P:(s + 1) * P],
                                         h_ps[:, s * P:(s + 1) * P], AF.Silu,
                                         bias=rb1_sb[:], scale=1.0)
                    nc.tensor.matmul(
                        lg_ps[:, s, :], lhsT=h_sb[:, s * P:(s + 1) * P],
                        rhs=rw2a_sb[:], start=True, stop=True)

                # ---- xa_s = At8.T @ x8 (fp8 DoubleRow, 2x rate) ----
                xa_ps = []
                for half in range(2):
                    xp = ps_xa.tile([P, TT], F32, tag=f"xa{half}")
                    for th in range(2):
                        for j in range(KCP):
                            nc.tensor.matmul(
                                xp[:, th * TH:(th + 1) * TH],
                                lhsT=at8_sb[:, j, :,
                                            half * P:(half + 1) * P],
                                rhs=x8_sb[:, j, :, th * TH:(th + 1) * TH],
                                start=(j == 0), stop=(j == KCP - 1),
                                perf_mode=DR)
                    xa_ps.append(xp)

                # ---- top2 + softmax -> coeff (token-major), on DVE/ACT ----
                coeff_all = cfp.tile([P, TT // P, E], BF16, tag="coefball")
                for s in range(TT // P):
                    e_sb = smallp.tile([P, E], F32, tag=f"e{s % 2}")
                    nc.scalar.activation(e_sb[:], lg_ps[:, s, :], AF.Exp)
                    m1 = smallp.tile([P, 1], F32, tag="m1")
                    nc.vector.tensor_reduce(m1[:], e_sb[:],
                                            axis=mybir.AxisListType.X,
                                            op=ALU.max)
                    mask1 = smallp.tile([P, E], F32, tag="mask1")
                    nc.vector.tensor_scalar(mask1[:], e_sb[:], m1[:], None,
                                            op0=ALU.is_ge)
                    masked = smallp.tile([P, E], F32, tag="masked")
                    nc.vector.scalar_tensor_tensor(
                        masked[:], in0=mask1[:], scalar=NEG_BIG, in1=e_sb[:],
                        op0=ALU.mult, op1=ALU.add)
                    m2 = smallp.tile([P, 1], F32, tag="m2")
                    nc.vector.tensor_reduce(m2[:], masked[:],
                                            axis=mybir.AxisListType.X,
                                            op=ALU.max)
                    s12 = smallp.tile([P, 1], F32, tag="s12")
                    nc.vector.tensor_tensor(s12[:], m1[:], m2[:], op=ALU.add)
                    rs = smallp.tile([P, 1], F32, tag="rs")
                    nc.vector.reciprocal(rs[:], s12[:])
                    mask2 = smallp.tile([P, E], F32, tag="mask2")
                    nc.vector.tensor_scalar(mask2[:], e_sb[:], m2[:], None,
                                            op0=ALU.is_ge)
                    nc.vector.scalar_tensor_tensor(
                        coeff_all[:, s, :], in0=e_sb[:], scalar=rs[:],
                        in1=mask2[:], op0=ALU.mult, op1=ALU.mult)

                # ---- one PE transpose coeff [128,(4s,16e)] -> [64se,128t] ----
                ct_ps = ps_sm.tile([TT // P * E, P], BF16, tag="lgct")
                nc.tensor.transpose(ct_ps[:], coeff_all[:], ident_sb[:])
                ct_sb = cfp.tile([TT // P * E, P], BF16, tag="ctsb")
                nc.vector.tensor_copy(ct_sb[:], ct_ps[:])

                # ---- expand coeff.T rows to (e,r) rows (scaled /128);
                #      wxa8 = xa_s * cexp quantized to fp8e4 on DVE ----
                wxa_sb = cfp.tile([P, 2, TT], F8, tag="wxa")
                for half in range(2):
                    cx_ps = ps_sm.tile([P, TT], F32, tag="cx")
                    for s in range(TT // P):
                        nc.tensor.matmul(
                            cx_ps[:, s * P:(s + 1) * P],
                            lhsT=ex4_sb[:, s, half, :],
                            rhs=ct_sb[:], start=True, stop=True)
                    cx_sb = cfp.tile([P, TT], F32, tag=f"cxs{half}")
                    nc.vector.tensor_copy(cx_sb[:], cx_ps[:])
                    nc.vector.tensor_tensor(wxa_sb[:, half, :], xa_ps[half][:],
                                            cx_sb[:], op=ALU.mult)
                return xb_sb, wxa_sb

            def drain(ps, oc, t0, tt):
                o_sb = outp.tile([P, TT], BF16)
                # epilogue on ACT: keeps the PSUM drain off DVE, which
                # is busy with the next tile's top-2 chain.  The final
                # chunks drain in halves so the last out-DMA starts as
                # early as possible (shorter kernel tail).
                if tt == NT - 1 and oc == OC - 1:
                    nseg = 4
                elif tt == NT - 1 and oc >= OC - 3:
                    nseg = 2
                else:
                    nseg = 1
                sw = TT // nseg
                for sg in range(nseg):
                    nc.scalar.activation(o_sb[:, sg * sw:(sg + 1) * sw],
                                         ps[:, sg * sw:(sg + 1) * sw],
                                         AF.Identity,
                                         bias=bias_sb[:, oc:oc + 1],
                                         scale=1.0)
                    nc.sync.dma_start(
                        out=outT_d[oc * P:(oc + 1) * P,
                                   t0 + sg * sw:t0 + (sg + 1) * sw],
                        in_=o_sb[:, sg * sw:(sg + 1) * sw])

            def back(tt, state):
                # base + delta accumulated per 128-row output chunk; chunks
                # processed in pairs so the bf16->fp8 PE mode switch for the
                # DoubleRow delta matmuls is paid once per pair
                t0 = tt * TT
                xb_sb, wxa_sb = state

                def base_mms(oc):
                    ps = ps_out.tile([P, TT], F32, tag="out")
                    for k in range(KC):
                        nc.tensor.matmul(ps[:], lhsT=wt_sb[:, oc, k, :],
                                         rhs=xb_sb[:, k, :],
                                         start=(k == 0), stop=False)
                    return ps

                def delta_mms(ps, oc):
                    for th in range(2):
                        nc.tensor.matmul(
                            ps[:, th * TH:(th + 1) * TH],
                            lhsT=bf8_sb[:, :, oc * P:(oc + 1) * P],
                            rhs=wxa_sb[:, :, th * TH:(th + 1) * TH],
                            start=False, stop=True,
                            perf_mode=DR, skip_group_check=True)

                # chunks in triples (PSUM bufs=3): the bf16<->fp8 switch for
                # the DoubleRow deltas is paid once per triple
                groups = [(0, 1, 2), (3, 4, 5), (6, 7, 8), (9, 10, 11),
                          (12, 13, 14), (15,)]
                for gi, ocs in enumerate(groups):
                    if tt == NT - 1 and gi >= len(groups) - 2:
                        # end of the run: finish each chunk (delta + drain)
                        # immediately so the tail after the last matmul
                        # covers only a fraction of one chunk
                        for oc in ocs:
                            ps = base_mms(oc)
                            delta_mms(ps, oc)
                            drain(ps, oc, t0, tt)
                    else:
                        pss = [base_mms(oc) for oc in ocs]
                        for i, oc in enumerate(ocs):
                            delta_mms(pss[i], oc)
                        for i, oc in enumerate(ocs):
                            drain(pss[i], oc, t0, tt)

            # one-tile-deep software pipeline: front(j+1) fills the PE
            # while back(j) waits on wt / PSUM drains
            states = {0: front(0)}
            for tt in range(NT):
                if tt + 1 < NT:
                    states[tt + 1] = front(tt + 1)
                back(tt, states.pop(tt))

    nc.compile()
    return nc


_CACHE = {}


def _get_nc(n_core: int):
    if n_core not in _CACHE:
        _CACHE[n_core] = _build(n_core)
    return _CACHE[n_core]


def _prep_in_maps(x, W, bias, rw1, rb1, rw2, rb2, A, B, gates):
    x, W, bias, rw1, rb1, rw2, rb2, A, B, gates = (
        np.asarray(v) for v in (x, W, bias, rw1, rb1, rw2, rb2, A, B, gates))
    xf = np.ascontiguousarray(x.reshape(-1, D).astype(np.float32))
    n = xf.shape[0]
    assert n % N_CORES == 0
    n_core = n // N_CORES

    bf16 = ml_dtypes.bfloat16
    f8 = mybir.dt.np(mybir.dt.float8e4)
    xT = np.ascontiguousarray(xf.T)                      # [D, n] f32
    xTb = xT.astype(bf16)
    xT8 = xT.astype(f8)
    # W.T blocked i-major [OC, 128i, KC, 128o]
    wt = np.ascontiguousarray(
        W.astype(np.float32).T.reshape(KC, P, OC, P).transpose(2, 1, 0, 3)
    ).astype(bf16)
    # A.T scaled + k-paired: [P, KCP, 2, ER]
    a32t = (A.astype(np.float32) * SC_A).reshape(ER, D).T  # [D, ER]
    at8 = np.ascontiguousarray(
        a32t.reshape(KCP, 2, P, ER).transpose(2, 0, 1, 3)).astype(f8)
    # B rows scaled + er-paired: [P, 2, D]
    bfl = (B.astype(np.float32) * SC_B).transpose(0, 2, 1).reshape(ER, D)
    bf8 = np.ascontiguousarray(
        bfl.reshape(2, P, D).transpose(1, 0, 2)).astype(f8)
    # router W1.T packed [P, KC, HID]
    rw1t = np.ascontiguousarray(
        rw1.astype(np.float32).T.reshape(KC, P, HID).transpose(1, 0, 2)
    ).astype(bf16)
    rb1c = np.ascontiguousarray(rb1.astype(np.float32).reshape(HID, 1))
    rw2a = np.concatenate(
        [rw2[:E].astype(np.float32).T,
         (rb2[:E].astype(np.float32) + gates.astype(np.float32))[None, :]],
        axis=0)
    rw2a = np.ascontiguousarray(rw2a)
    biaspp = np.ascontiguousarray(
        bias.astype(np.float32).reshape(OC, P).T)
    # ex4[se, s, h, j] = SC_E iff se's s-block matches and expert(se) owns
    # lora row h*128+j  (se = s*E + e)
    ex4 = np.zeros((4 * E, 4, 2, P), np.float32)
    for s in range(4):
        for e in range(E):
            for j in range(ER):
                ex4[s * E + e, s, j // P, j % P] = (
                    SC_E if j // R == e else 0.0)
    ex4 = ex4.astype(bf16)
    ident = np.eye(P, dtype=np.float32).astype(bf16)

    shared = {"wt": wt, "at8": at8, "bf8": bf8, "rw1t": rw1t, "rb1": rb1c,
              "rw2a": rw2a, "biaspp": biaspp, "ex4": ex4, "ident": ident}
    NT = n_core // TT
    in_maps = []
    for c in range(N_CORES):
        sl = slice(c * n_core, (c + 1) * n_core)
        xc = (xTb[:, sl].reshape(KC, P, NT, TT)
              .transpose(2, 1, 0, 3))
        xc8 = (xT8[:, sl].reshape(KCP, 2, P, NT, TT)
               .transpose(3, 2, 0, 1, 4))
        in_maps.append({"xbf": np.ascontiguousarray(xc),
                        "x8": np.ascontiguousarray(xc8), **shared})
    return in_maps, n_core


def kernel(x, W, bias, rw1, rb1, rw2, rb2, A, B, gates):
    lead = x.shape[:-1]
    in_maps, n_core = _prep_in_maps(x, W, bias, rw1, rb1, rw2, rb2, A, B,
                                    gates)
    n = n_core * N_CORES
    nc = _get_nc(n_core)
    res = None
    for attempt in range(3):
        try:
            res = run_bass_kernel_spmd(nc, in_maps,
                                       core_ids=list(range(N_CORES)))
            break
        except Exception:
            # sporadic NRT_EXEC_UNIT_UNRECOVERABLE on a fresh NEFF; retry
            if attempt == 2:
                raise
            import time as _time

            _time.sleep(10)

    out = np.empty((n, D), np.float32)
    for c in range(N_CORES):
        out[c * n_core:(c + 1) * n_core] = (
            res.results[c]["outT"].astype(np.float32).T)
    return out.reshape(*lead, D)


# revision 27
# speedup vs baseline: 1.0273x; 1.0273x over previous
"""Additive LoRA adapter (MoE-routed) forward — Trainium2, 8 NeuronCores.

Data-parallel over tokens: each core gets n/8 tokens, weights replicated.
Forward only => no collectives.

Per-core compute (feature-major / "transposed activations" layout):
  - base:   out.T[o,t] += sum_k Wt[k,o-block].T @ xbf[k,t]     (bf16 matmul)
  - router: h[hid,t] = silu(rw1t.T @ xf32 + rb1)               (bf16 matmul)
            logits[t,e] = (h_aug[.,t-block]).T @ rw2_aug       (fp32, bias+gates
            via the appended ones-row of h_aug)
  - top2 + softmax on VectorE via exp / two maxes / masks
  - coeff.T via PE transpose, expanded to (e,r)-rows via a scaled 0/1
    expand matmul (entries ALPHA/128 fold the fp8 scales back out)
  - xa.T[(e,r),t] = At8.T @ x8 as fp8e4 DoubleRow matmuls (2x PE rate);
    A is pre-scaled by 32 so fp8e4 stays out of the subnormal range
  - wxa8 = xa_s * (coeff/128) quantized to fp8e4 on DVE
  - delta accumulated into the same PSUM groups as base via fp8 DoubleRow
    against B pre-scaled by 4 (32 * 4 / 128 = 1 => unscaled delta).
    Output chunks are processed in PAIRS so the bf16<->fp8 PE mode switch
    (~0.25us each way) is paid once per pair, not once per chunk.
  - router h stays bf16: fp8 logit noise flips ~7% of top-2 picks, which
    costs ~1.5e-2 rel err; bf16 keeps it at ~5e-3.
Host pre-transposes/quantizes so every matmul operand is a natural
(contraction-on-partitions) SBUF load; weights/x are packed so every DMA
moves >=2KB contiguous per partition (small-line DMAs starve startup).
Output is produced transposed in bf16 and un-transposed on the host.
"""
import sys

sys.path.insert(0, "/opt/trn_rl_repo")

import numpy as np
import ml_dtypes

from concourse import bacc, tile, mybir
from concourse.bass_utils import run_bass_kernel_spmd

N_CORES = 8
D = 2048          # d_in == d_out
E = 16            # populated experts
R = 16            # lora rank
ER = E * R        # 256
HID = 64          # router hidden
P = 128           # partitions
KC = D // P       # 16 contraction chunks
KCP = KC // 2     # 8 contraction chunk-pairs (fp8 DoubleRow)
OC = D // P       # 16 output chunks
TT = 512          # token tile
TH = TT // 2      # fp8 DoubleRow moving-token chunk (256)
ALPHA = 1.0
WARM_MM = 58      # PE warm-up matmuls covering DMA spin-up

# fp8 scaling: A8 = 32*A, B8 = 4*B, expand entries ALPHA/128 => delta unscaled
SC_A = 32.0
SC_B = 4.0
SC_E = ALPHA / (SC_A * SC_B)

F32 = mybir.dt.float32
BF16 = mybir.dt.bfloat16
F8 = mybir.dt.float8e4
AF = mybir.ActivationFunctionType
ALU = mybir.AluOpType
DR = mybir.MatmulPerfMode.DoubleRow
NEG_BIG = -1.0e30


def _build(n_core: int):
    NT = n_core // TT
    nc = bacc.Bacc("TRN2", target_bir_lowering=False, debug=False,
                   num_devices=N_CORES)

    # x bf16 blocked [NT, P, KC, TT] -> 4KB-line DMAs (k-quarters)
    xbf_d = nc.dram_tensor("xbf", [NT, P, KC, TT], BF16,
                           kind="ExternalInput").ap()
    # x fp8 k-paired [NT, P, KCP, 2, TT] -> DoubleRow moving operand
    x8_d = nc.dram_tensor("x8", [NT, P, KCP, 2, TT], F8,
                          kind="ExternalInput").ap()
    # W.T blocked i-major: [OC, 128i, KC, 128o] -> contiguous 4KB per
    # partition per oc-slab
    wt_d = nc.dram_tensor("wt", [OC, P, KC, P], BF16, kind="ExternalInput").ap()
    # A.T fp8 k-paired [P, KCP, 2, ER] (scaled by SC_A) -> one 4KB-line DMA
    at8_d = nc.dram_tensor("at8", [P, KCP, 2, ER], F8,
                           kind="ExternalInput").ap()
    # B rows fp8 er-paired [P, 2, D] (scaled by SC_B) -> one 4KB-line DMA
    bf8_d = nc.dram_tensor("bf8", [P, 2, D], F8, kind="ExternalInput").ap()
    # router W1.T packed [P, KC, HID] -> one 2KB-line DMA
    rw1t_d = nc.dram_tensor("rw1t", [P, KC, HID], BF16,
                            kind="ExternalInput").ap()
    rb1_d = nc.dram_tensor("rb1", [HID, 1], F32, kind="ExternalInput").ap()
    rw2a_d = nc.dram_tensor("rw2a", [HID + 1, E], BF16,
                            kind="ExternalInput").ap()
    bias_d = nc.dram_tensor("biaspp", [P, OC], F32, kind="ExternalInput").ap()
    # per-s-chunk expand blocks [64se, 4s, 2half, 128j] (entries ALPHA/128)
    ex4_d = nc.dram_tensor("ex4", [4 * E, 4, 2, P], BF16,
                           kind="ExternalInput").ap()
    ident_d = nc.dram_tensor("ident", [P, P], BF16, kind="ExternalInput").ap()
    outT_d = nc.dram_tensor("outT", [D, n_core], BF16,
                            kind="ExternalOutput").ap()

    with tile.TileContext(nc) as tc:
        with (
            tc.tile_pool(name="const", bufs=1) as constp,
            tc.tile_pool(name="wres", bufs=1) as wres,
            tc.tile_pool(name="xb", bufs=3) as xbp,
            tc.tile_pool(name="x8p", bufs=3) as x8p,
            tc.tile_pool(name="hp", bufs=2) as hp,
            tc.tile_pool(name="small", bufs=4) as smallp,
            tc.tile_pool(name="cf", bufs=2) as cfp,
            tc.tile_pool(name="outp", bufs=4) as outp,
            tc.tile_pool(name="ps_out", bufs=3, space="PSUM") as ps_out,
            tc.tile_pool(name="ps_xa", bufs=1, space="PSUM") as ps_xa,
            tc.tile_pool(name="ps_h", bufs=1, space="PSUM") as ps_h,
            tc.tile_pool(name="ps_sm", bufs=1, space="PSUM") as ps_sm,
        ):
            # ---- startup-critical loads first: router W1, A, x tile 0 ----
            rw1t_sb = constp.tile([P, KC, HID], BF16)
            nc.sync.dma_start(out=rw1t_sb[:], in_=rw1t_d[:])
            at8_sb = wres.tile([P, KCP, 2, ER], F8)
            nc.sync.dma_start(out=at8_sb[:], in_=at8_d[:])
            rb1_sb = constp.tile([HID, 1], F32)
            nc.sync.dma_start(out=rb1_sb[:], in_=rb1_d[:])
            rw2a_sb = constp.tile([HID + 1, E], BF16)
            nc.sync.dma_start(out=rw2a_sb[:], in_=rw2a_d[:])

            def load_x_tile(tt):
                xb_sb = xbp.tile([P, KC, TT], BF16)
                for q in range(4):
                    nc.sync.dma_start(out=xb_sb[:, 4 * q:4 * q + 4, :],
                                      in_=xbf_d[tt, :, 4 * q:4 * q + 4, :])
                x8_sb = x8p.tile([P, KCP, 2, TT], F8)
                for hseg in range(2):
                    nc.sync.dma_start(
                        out=x8_sb[:, 4 * hseg:4 * hseg + 4, :, :],
                        in_=x8_d[tt, :, 4 * hseg:4 * hseg + 4, :, :])
                return xb_sb, x8_sb

            x_tile0 = load_x_tile(0)
            ex4_sb = constp.tile([4 * E, 4, 2, P], BF16)
            nc.sync.dma_start(out=ex4_sb[:], in_=ex4_d[:])
            ident_sb = constp.tile([P, P], BF16)
            nc.sync.dma_start(out=ident_sb[:], in_=ident_d[:])

            # ---- resident weights; tile-1 x hoisted ahead of W so the
            # tile boundary never starves ----
            x_tile1 = load_x_tile(1) if NT > 1 else None
            bias_sb = constp.tile([P, OC], F32)
            nc.sync.dma_start(out=bias_sb[:], in_=bias_d[:])
            bf8_sb = wres.tile([P, 2, D], F8)
            nc.sync.dma_start(out=bf8_sb[:], in_=bf8_d[:])
            x_tile2 = None
            wt_sb = wres.tile([P, OC, KC, P], BF16)
            for oc in range(OC):
                nc.sync.dma_start(
                    out=wt_sb[:, oc, :, :],
                    in_=wt_d[oc, :, :, :])
                if oc == 6 and NT > 2:
                    # slot tile-2's x into the W stream: lands before the
                    # pipeline needs it, without delaying early W chunks
                    x_tile2 = load_x_tile(2)

            def front(tt):
                # router + xa + top2 + wxa: everything that does NOT need W.
                # Emitted one tile ahead of back() so the PE always has
                # W-independent work while wt/xb stream in.
                if tt == 0:
                    xb_sb, x8_sb = x_tile0
                elif tt == 1:
                    xb_sb, x8_sb = x_tile1
                elif tt == 2:
                    xb_sb, x8_sb = x_tile2
                else:
                    xb_sb, x8_sb = load_x_tile(tt)

                # ---- router hidden: h = silu(rw1t.T @ x + rb1) (bf16 mm) ----
                h_ps = ps_h.tile([HID, TT], F32)
                if tt == 0:
                    # dummy matmuls on a memset scratch tile (no DMA deps):
                    # fill the DMA spin-up so the PE is busy and HAM
                    # stays at 2.4GHz when real work arrives
                    scr_sb = constp.tile([P, TT], BF16)
                    nc.vector.memset(scr_sb[:], 1.0)
                    warm_ps = ps_out.tile([P, TT], F32, tag="out")
                    for _ in range(WARM_MM):
                        nc.tensor.matmul(warm_ps[:], lhsT=scr_sb[:, 0:P],
                                         rhs=scr_sb[:], start=True,
                                         stop=True)
                for k in range(KC):
                    nc.tensor.matmul(h_ps[:], lhsT=rw1t_sb[:, k, :],
                                     rhs=xb_sb[:, k, :],
                                     start=(k == 0), stop=(k == KC - 1))
                h_sb = hp.tile([HID + 1, TT], BF16)
                nc.vector.memset(h_sb[HID:HID + 1, :], 1.0)

                # ---- silu + logits per 128-token sub-chunk (bf16 mm) ----
                lg_ps = ps_sm.tile([P, TT // P, E], F32, tag="lgct")
                for s in range(TT // P):
                    nc.scalar.activation(h_sb[0:HID, s * P:(s + 1) * P],
                                         h_ps[:, s * P:(s + 1) * P], AF.Silu,
                                         bias=rb1_sb[:], scale=1.0)
                    nc.tensor.matmul(
                        lg_ps[:, s, :], lhsT=h_sb[:, s * P:(s + 1) * P],
                        rhs=rw2a_sb[:], start=True, stop=True)

                # ---- xa_s = At8.T @ x8 (fp8 DoubleRow, 2x rate) ----
                xa_ps = []
                for half in range(2):
                    xp = ps_xa.tile([P, TT], F32, tag=f"xa{half}")
                    for th in range(2):
                        for j in range(KCP):
                            nc.tensor.matmul(
                                xp[:, th * TH:(th + 1) * TH],
                                lhsT=at8_sb[:, j, :,
                                            half * P:(half + 1) * P],
                                rhs=x8_sb[:, j, :, th * TH:(th + 1) * TH],
                                start=(j == 0), stop=(j == KCP - 1),
                                perf_mode=DR)
                    xa_ps.append(xp)

                # ---- top2 + softmax -> coeff (token-major), on DVE/ACT ----
                coeff_all = cfp.tile([P, TT // P, E], BF16, tag="coefball")
                for s in range(TT // P):
                    e_sb = smallp.tile([P, E], F32, tag=f"e{s % 2}")
                    nc.scalar.activation(e_sb[:], lg_ps[:, s, :], AF.Exp)
                    m1 = smallp.tile([P, 1], F32, tag="m1")
                    nc.vector.tensor_reduce(m1[:], e_sb[:],
                                            axis=mybir.AxisListType.X,
                                            op=ALU.max)
                    mask1 = smallp.tile([P, E], F32, tag="mask1")
                    nc.vector.tensor_scalar(mask1[:], e_sb[:], m1[:], None,
                                            op0=ALU.is_ge)
                    masked = smallp.tile([P, E], F32, tag="masked")
                    nc.vector.scalar_tensor_tensor(
                        masked[:], in0=mask1[:], scalar=NEG_BIG, in1=e_sb[:],
                        op0=ALU.mult, op1=ALU.add)
                    m2 = smallp.tile([P, 1], F32, tag="m2")
                    nc.vector.tensor_reduce(m2[:], masked[:],
                                            axis=mybir.AxisListType.X,
                                            op=ALU.max)
                    s12 = smallp.tile([P, 1], F32, tag="s12")
                    nc.vector.tensor_tensor(s12[:], m1[:], m2[:], op=ALU.add)
                    rs = smallp.tile([P, 1], F32, tag="rs")
                    nc.vector.reciprocal(rs[:], s12[:])
                    mask2 = smallp.tile([P, E], F32, tag="mask2")
                    nc.vector.tensor_scalar(mask2[:], e_sb[:], m2[:], None,
                                            op0=ALU.is_ge)
                    nc.vector.scalar_tensor_tensor(
                        coeff_all[:, s, :], in0=e_sb[:], scalar=rs[:],
                        in1=mask2[:], op0=ALU.mult, op1=ALU.mult)

                # ---- one PE transpose coeff [128,(4s,16e)] -> [64se,128t] ----
                ct_ps = ps_sm.tile([TT // P * E, P], BF16, tag="lgct")
                nc.tensor.transpose(ct_ps[:], coeff_all[:], ident_sb[:])
                ct_sb = cfp.tile([TT // P * E, P], BF16, tag="ctsb")
                nc.vector.tensor_copy(ct_sb[:], ct_ps[:])

                # ---- expand coeff.T rows to (e,r) rows (scaled /128);
                #      wxa8 = xa_s * cexp quantized to fp8e4 on DVE ----
                wxa_sb = cfp.tile([P, 2, TT], F8, tag="wxa")
                for half in range(2):
                    cx_ps = ps_sm.tile([P, TT], F32, tag="cx")
                    for s in range(TT // P):
                        nc.tensor.matmul(
                            cx_ps[:, s * P:(s + 1) * P],
                            lhsT=ex4_sb[:, s, half, :],
                            rhs=ct_sb[:], start=True, stop=True)
                    cx_sb = cfp.tile([P, TT], F32, tag=f"cxs{half}")
                    nc.vector.tensor_copy(cx_sb[:], cx_ps[:])
                    nc.vector.tensor_tensor(wxa_sb[:, half, :], xa_ps[half][:],
                                            cx_sb[:], op=ALU.mult)
                return xb_sb, wxa_sb

            def drain(ps, oc, t0, tt):
                o_sb = outp.tile([P, TT], BF16)
                # epilogue on ACT: keeps the PSUM drain off DVE, which
                # is busy with the next tile's top-2 chain.  The final
                # chunks drain in halves so the last out-DMA starts as
                # early as possible (shorter kernel tail).
                nseg = 2 if (tt == NT - 1 and oc >= OC - 2) else 1
                sw = TT // nseg
                for sg in range(nseg):
                    nc.scalar.activation(o_sb[:, sg * sw:(sg + 1) * sw],
                                         ps[:, sg * sw:(sg + 1) * sw],
                                         AF.Identity,
                                         bias=bias_sb[:, oc:oc + 1],
                                         scale=1.0)
                    nc.sync.dma_start(
                        out=outT_d[oc * P:(oc + 1) * P,
                                   t0 + sg * sw:t0 + (sg + 1) * sw],
                        in_=o_sb[:, sg * sw:(sg + 1) * sw])

            def back(tt, state):
                # base + delta accumulated per 128-row output chunk; chunks
                # processed in pairs so the bf16->fp8 PE mode switch for the
                # DoubleRow delta matmuls is paid once per pair
                t0 = tt * TT
                xb_sb, wxa_sb = state

                def base_mms(oc):
                    ps = ps_out.tile([P, TT], F32, tag="out")
                    for k in range(KC):
                        nc.tensor.matmul(ps[:], lhsT=wt_sb[:, oc, k, :],
                                         rhs=xb_sb[:, k, :],
                                         start=(k == 0), stop=False)
                    return ps

                def delta_mms(ps, oc):
                    for th in range(2):
                        nc.tensor.matmul(
                            ps[:, th * TH:(th + 1) * TH],
                            lhsT=bf8_sb[:, :, oc * P:(oc + 1) * P],
                            rhs=wxa_sb[:, :, th * TH:(th + 1) * TH],
                            start=False, stop=True,
                            perf_mode=DR, skip_group_check=True)

                # chunks in pairs (PSUM bufs=3 leaves one spare so the next
                # pair never stalls on this pair's drains): the bf16<->fp8
                # switch for the DoubleRow deltas is paid once per pair
                for op in range(OC // 2):
                    ocs = (2 * op, 2 * op + 1)
                    if tt == NT - 1 and op == OC // 2 - 1:
                        # last pair of the run: finish each chunk (delta +
                        # drain) immediately so the final drain+DMA tail
                        # after the last matmul covers just half a chunk
                        for oc in ocs:
                            ps = base_mms(oc)
                            delta_mms(ps, oc)
                            drain(ps, oc, t0, tt)
                    else:
                        pss = [base_mms(oc) for oc in ocs]
                        for half in range(2):
                            delta_mms(pss[half], ocs[half])
                        for half in range(2):
                            drain(pss[half], ocs[half], t0, tt)

            # one-tile-deep software pipeline: front(j+1) fills the PE
            # while back(j) waits on wt / PSUM drains
            states = {0: front(0)}
            for tt in range(NT):
                if tt + 1 < NT:
                    states[tt + 1] = front(tt + 1)
                back(tt, states.pop(tt))

    nc.compile()
    return nc


_CACHE = {}


def _get_nc(n_core: int):
    if n_core not in _CACHE:
        _CACHE[n_core] = _build(n_core)
    return _CACHE[n_core]


def _prep_in_maps(x, W, bias, rw1, rb1, rw2, rb2, A, B, gates):
    x, W, bias, rw1, rb1, rw2, rb2, A, B, gates = (
        np.asarray(v) for v in (x, W, bias, rw1, rb1, rw2, rb2, A, B, gates))
    xf = np.ascontiguousarray(x.reshape(-1, D).astype(np.float32))
    n = xf.shape[0]
    assert n % N_CORES == 0
    n_core = n // N_CORES

    bf16 = ml_dtypes.bfloat16
    f8 = mybir.dt.np(mybir.dt.float8e4)
    xT = np.ascontiguousarray(xf.T)                      # [D, n] f32
    xTb = xT.astype(bf16)
    xT8 = xT.astype(f8)
    # W.T blocked i-major [OC, 128i, KC, 128o]
    wt = np.ascontiguousarray(
        W.astype(np.float32).T.reshape(KC, P, OC, P).transpose(2, 1, 0, 3)
    ).astype(bf16)
    # A.T scaled + k-paired: [P, KCP, 2, ER]
    a32t = (A.astype(np.float32) * SC_A).reshape(ER, D).T  # [D, ER]
    at8 = np.ascontiguousarray(
        a32t.reshape(KCP, 2, P, ER).transpose(2, 0, 1, 3)).astype(f8)
    # B rows scaled + er-paired: [P, 2, D]
    bfl = (B.astype(np.float32) * SC_B).transpose(0, 2, 1).reshape(ER, D)
    bf8 = np.ascontiguousarray(
        bfl.reshape(2, P, D).transpose(1, 0, 2)).astype(f8)
    # router W1.T packed [P, KC, HID]
    rw1t = np.ascontiguousarray(
        rw1.astype(np.float32).T.reshape(KC, P, HID).transpose(1, 0, 2)
    ).astype(bf16)
    rb1c = np.ascontiguousarray(rb1.astype(np.float32).reshape(HID, 1))
    rw2a = np.concatenate(
        [rw2[:E].astype(np.float32).T,
         (rb2[:E].astype(np.float32) + gates.astype(np.float32))[None, :]],
        axis=0)
    rw2a = np.ascontiguousarray(rw2a).astype(bf16)
    biaspp = np.ascontiguousarray(
        bias.astype(np.float32).reshape(OC, P).T)
    # ex4[se, s, h, j] = SC_E iff se's s-block matches and expert(se) owns
    # lora row h*128+j  (se = s*E + e)
    ex4 = np.zeros((4 * E, 4, 2, P), np.float32)
    for s in range(4):
        for e in range(E):
            for j in range(ER):
                ex4[s * E + e, s, j // P, j % P] = (
                    SC_E if j // R == e else 0.0)
    ex4 = ex4.astype(bf16)
    ident = np.eye(P, dtype=np.float32).astype(bf16)

    shared = {"wt": wt, "at8": at8, "bf8": bf8, "rw1t": rw1t, "rb1": rb1c,
              "rw2a": rw2a, "biaspp": biaspp, "ex4": ex4, "ident": ident}
    NT = n_core // TT
    in_maps = []
    for c in range(N_CORES):
        sl = slice(c * n_core, (c + 1) * n_core)
        xc = (xTb[:, sl].reshape(KC, P, NT, TT)
              .transpose(2, 1, 0, 3))
        xc8 = (xT8[:, sl].reshape(KCP, 2, P, NT, TT)
               .transpose(3, 2, 0, 1, 4))
        in_maps.append({"xbf": np.ascontiguousarray(xc),
                        "x8": np.ascontiguousarray(xc8), **shared})
    return in_maps, n_core


def kernel(x, W, bias, rw1, rb1, rw2, rb2, A, B, gates):
    lead = x.shape[:-1]
    in_maps, n_core = _prep_in_maps(x, W, bias, rw1, rb1, rw2, rb2, A, B,
                                    gates)
    n = n_core * N_CORES
    nc = _get_nc(n_core)
    res = None
    for attempt in range(3):
        try:
            res = run_bass_kernel_spmd(nc, in_maps,
                                       core_ids=list(range(N_CORES)))
            break
        except Exception:
            # sporadic NRT_EXEC_UNIT_UNRECOVERABLE on a fresh NEFF; retry
            if attempt == 2:
                raise
            import time as _time

            _time.sleep(10)

    out = np.empty((n, D), np.float32)
    for c in range(N_CORES):
        out[c * n_core:(c + 1) * n_core] = (
            res.results[c]["outT"].astype(np.float32).T)
    return out.reshape(*lead, D)


# revision 28
# speedup vs baseline: 1.0285x; 1.0012x over previous
"""Additive LoRA adapter (MoE-routed) forward — Trainium2, 8 NeuronCores.

Data-parallel over tokens: each core gets n/8 tokens, weights replicated.
Forward only => no collectives.

Per-core compute (feature-major / "transposed activations" layout):
  - base:   out.T[o,t] += sum_k Wt[k,o-block].T @ xbf[k,t]     (bf16 matmul)
  - router: h[hid,t] = silu(rw1t.T @ xf32 + rb1)               (bf16 matmul)
            logits[t,e] = (h_aug[.,t-block]).T @ rw2_aug       (fp32, bias+gates
            via the appended ones-row of h_aug)
  - top2 + softmax on VectorE via exp / two maxes / masks
  - coeff.T via PE transpose, expanded to (e,r)-rows via a scaled 0/1
    expand matmul (entries ALPHA/128 fold the fp8 scales back out)
  - xa.T[(e,r),t] = At8.T @ x8 as fp8e4 DoubleRow matmuls (2x PE rate);
    A is pre-scaled by 32 so fp8e4 stays out of the subnormal range
  - wxa8 = xa_s * (coeff/128) quantized to fp8e4 on DVE
  - delta accumulated into the same PSUM groups as base via fp8 DoubleRow
    against B pre-scaled by 4 (32 * 4 / 128 = 1 => unscaled delta).
    Output chunks are processed in PAIRS so the bf16<->fp8 PE mode switch
    (~0.25us each way) is paid once per pair, not once per chunk.
  - router h stays bf16: fp8 logit noise flips ~7% of top-2 picks, which
    costs ~1.5e-2 rel err; bf16 keeps it at ~5e-3.
Host pre-transposes/quantizes so every matmul operand is a natural
(contraction-on-partitions) SBUF load; weights/x are packed so every DMA
moves >=2KB contiguous per partition (small-line DMAs starve startup).
Output is produced transposed in bf16 and un-transposed on the host.
"""
import sys

sys.path.insert(0, "/opt/trn_rl_repo")

import numpy as np
import ml_dtypes

from concourse import bacc, tile, mybir
from concourse.bass_utils import run_bass_kernel_spmd

N_CORES = 8
D = 2048          # d_in == d_out
E = 16            # populated experts
R = 16            # lora rank
ER = E * R        # 256
HID = 64          # router hidden
P = 128           # partitions
KC = D // P       # 16 contraction chunks
KCP = KC // 2     # 8 contraction chunk-pairs (fp8 DoubleRow)
OC = D // P       # 16 output chunks
TT = 512          # token tile
TH = TT // 2      # fp8 DoubleRow moving-token chunk (256)
ALPHA = 1.0
WARM_MM = 58      # PE warm-up matmuls covering DMA spin-up

# fp8 scaling: A8 = 32*A, B8 = 4*B, expand entries ALPHA/128 => delta unscaled
SC_A = 32.0
SC_B = 4.0
SC_E = ALPHA / (SC_A * SC_B)

F32 = mybir.dt.float32
BF16 = mybir.dt.bfloat16
F8 = mybir.dt.float8e4
AF = mybir.ActivationFunctionType
ALU = mybir.AluOpType
DR = mybir.MatmulPerfMode.DoubleRow
NEG_BIG = -1.0e30


def _build(n_core: int):
    NT = n_core // TT
    nc = bacc.Bacc("TRN2", target_bir_lowering=False, debug=False,
                   num_devices=N_CORES)

    # x bf16 blocked [NT, P, KC, TT] -> 4KB-line DMAs (k-quarters)
    xbf_d = nc.dram_tensor("xbf", [NT, P, KC, TT], BF16,
                           kind="ExternalInput").ap()
    # x fp8 k-paired [NT, P, KCP, 2, TT] -> DoubleRow moving operand
    x8_d = nc.dram_tensor("x8", [NT, P, KCP, 2, TT], F8,
                          kind="ExternalInput").ap()
    # W.T blocked i-major: [OC, 128i, KC, 128o] -> contiguous 4KB per
    # partition per oc-slab
    wt_d = nc.dram_tensor("wt", [OC, P, KC, P], BF16, kind="ExternalInput").ap()
    # A.T fp8 k-paired [P, KCP, 2, ER] (scaled by SC_A) -> one 4KB-line DMA
    at8_d = nc.dram_tensor("at8", [P, KCP, 2, ER], F8,
                           kind="ExternalInput").ap()
    # B rows fp8 er-paired [P, 2, D] (scaled by SC_B) -> one 4KB-line DMA
    bf8_d = nc.dram_tensor("bf8", [P, 2, D], F8, kind="ExternalInput").ap()
    # router W1.T packed [P, KC, HID] -> one 2KB-line DMA
    rw1t_d = nc.dram_tensor("rw1t", [P, KC, HID], BF16,
                            kind="ExternalInput").ap()
    rb1_d = nc.dram_tensor("rb1", [HID, 1], F32, kind="ExternalInput").ap()
    rw2a_d = nc.dram_tensor("rw2a", [HID + 1, E], BF16,
                            kind="ExternalInput").ap()
    bias_d = nc.dram_tensor("biaspp", [P, OC], F32, kind="ExternalInput").ap()
    # per-s-chunk expand blocks [64se, 4s, 2half, 128j] (entries ALPHA/128)
    ex4_d = nc.dram_tensor("ex4", [4 * E, 4, 2, P], BF16,
                           kind="ExternalInput").ap()
    ident_d = nc.dram_tensor("ident", [P, P], BF16, kind="ExternalInput").ap()
    outT_d = nc.dram_tensor("outT", [D, n_core], BF16,
                            kind="ExternalOutput").ap()

    with tile.TileContext(nc) as tc:
        with (
            tc.tile_pool(name="const", bufs=1) as constp,
            tc.tile_pool(name="wres", bufs=1) as wres,
            tc.tile_pool(name="xb", bufs=3) as xbp,
            tc.tile_pool(name="x8p", bufs=3) as x8p,
            tc.tile_pool(name="hp", bufs=2) as hp,
            tc.tile_pool(name="small", bufs=4) as smallp,
            tc.tile_pool(name="cf", bufs=2) as cfp,
            tc.tile_pool(name="outp", bufs=4) as outp,
            tc.tile_pool(name="ps_out", bufs=3, space="PSUM") as ps_out,
            tc.tile_pool(name="ps_xa", bufs=1, space="PSUM") as ps_xa,
            tc.tile_pool(name="ps_h", bufs=1, space="PSUM") as ps_h,
            tc.tile_pool(name="ps_sm", bufs=1, space="PSUM") as ps_sm,
        ):
            # ---- startup-critical loads first: router W1, A, x tile 0 ----
            rw1t_sb = constp.tile([P, KC, HID], BF16)
            nc.sync.dma_start(out=rw1t_sb[:], in_=rw1t_d[:])
            at8_sb = wres.tile([P, KCP, 2, ER], F8)
            nc.sync.dma_start(out=at8_sb[:], in_=at8_d[:])
            rb1_sb = constp.tile([HID, 1], F32)
            nc.sync.dma_start(out=rb1_sb[:], in_=rb1_d[:])
            rw2a_sb = constp.tile([HID + 1, E], BF16)
            nc.sync.dma_start(out=rw2a_sb[:], in_=rw2a_d[:])

            def load_x_tile(tt):
                xb_sb = xbp.tile([P, KC, TT], BF16)
                for q in range(4):
                    nc.sync.dma_start(out=xb_sb[:, 4 * q:4 * q + 4, :],
                                      in_=xbf_d[tt, :, 4 * q:4 * q + 4, :])
                x8_sb = x8p.tile([P, KCP, 2, TT], F8)
                for hseg in range(2):
                    nc.sync.dma_start(
                        out=x8_sb[:, 4 * hseg:4 * hseg + 4, :, :],
                        in_=x8_d[tt, :, 4 * hseg:4 * hseg + 4, :, :])
                return xb_sb, x8_sb

            x_tile0 = load_x_tile(0)
            ex4_sb = constp.tile([4 * E, 4, 2, P], BF16)
            nc.sync.dma_start(out=ex4_sb[:], in_=ex4_d[:])
            ident_sb = constp.tile([P, P], BF16)
            nc.sync.dma_start(out=ident_sb[:], in_=ident_d[:])

            # ---- resident weights; tile-1 x hoisted ahead of W so the
            # tile boundary never starves ----
            x_tile1 = load_x_tile(1) if NT > 1 else None
            x_tile2 = None
            bias_sb = constp.tile([P, OC], F32)
            bf8_sb = wres.tile([P, 2, D], F8)
            wt_sb = wres.tile([P, OC, KC, P], BF16)
            for oc in range(OC):
                nc.sync.dma_start(
                    out=wt_sb[:, oc, :, :],
                    in_=wt_d[oc, :, :, :])
                if oc == 2:
                    # bias/B dispatch after the first W chunks: back(0)
                    # needs wt0/wt1 several us before the first delta+drain
                    nc.sync.dma_start(out=bias_sb[:], in_=bias_d[:])
                    nc.sync.dma_start(out=bf8_sb[:], in_=bf8_d[:])
                if oc == 4 and NT > 2:
                    # slot tile-2's x into the W stream: lands before the
                    # pipeline needs it, without delaying early W chunks
                    x_tile2 = load_x_tile(2)

            def front(tt):
                # router + xa + top2 + wxa: everything that does NOT need W.
                # Emitted one tile ahead of back() so the PE always has
                # W-independent work while wt/xb stream in.
                if tt == 0:
                    xb_sb, x8_sb = x_tile0
                elif tt == 1:
                    xb_sb, x8_sb = x_tile1
                elif tt == 2:
                    xb_sb, x8_sb = x_tile2
                else:
                    xb_sb, x8_sb = load_x_tile(tt)

                # ---- router hidden: h = silu(rw1t.T @ x + rb1) (bf16 mm) ----
                h_ps = ps_h.tile([HID, TT], F32)
                if tt == 0:
                    # dummy matmuls on a memset scratch tile (no DMA deps):
                    # fill the DMA spin-up so the PE is busy and HAM
                    # stays at 2.4GHz when real work arrives
                    scr_sb = constp.tile([P, TT], BF16)
                    nc.vector.memset(scr_sb[:], 1.0)
                    warm_ps = ps_out.tile([P, TT], F32, tag="out")
                    for _ in range(WARM_MM):
                        nc.tensor.matmul(warm_ps[:], lhsT=scr_sb[:, 0:P],
                                         rhs=scr_sb[:], start=True,
                                         stop=True)
                for k in range(KC):
                    nc.tensor.matmul(h_ps[:], lhsT=rw1t_sb[:, k, :],
                                     rhs=xb_sb[:, k, :],
                                     start=(k == 0), stop=(k == KC - 1))
                h_sb = hp.tile([HID + 1, TT], BF16)
                nc.vector.memset(h_sb[HID:HID + 1, :], 1.0)

                # ---- silu + logits per 128-token sub-chunk (bf16 mm) ----
                lg_ps = ps_sm.tile([P, TT // P, E], F32, tag="lgct")
                for s in range(TT // P):
                    nc.scalar.activation(h_sb[0:HID, s * P:(s + 1) * P],
                                         h_ps[:, s * P:(s + 1) * P], AF.Silu,
                                         bias=rb1_sb[:], scale=1.0)
                    nc.tensor.matmul(
                        lg_ps[:, s, :], lhsT=h_sb[:, s * P:(s + 1) * P],
                        rhs=rw2a_sb[:], start=True, stop=True)

                # ---- xa_s = At8.T @ x8 (fp8 DoubleRow, 2x rate) ----
                xa_ps = []
                for half in range(2):
                    xp = ps_xa.tile([P, TT], F32, tag=f"xa{half}")
                    for th in range(2):
                        for j in range(KCP):
                            nc.tensor.matmul(
                                xp[:, th * TH:(th + 1) * TH],
                                lhsT=at8_sb[:, j, :,
                                            half * P:(half + 1) * P],
                                rhs=x8_sb[:, j, :, th * TH:(th + 1) * TH],
                                start=(j == 0), stop=(j == KCP - 1),
                                perf_mode=DR)
                    xa_ps.append(xp)

                # ---- top2 + softmax -> coeff (token-major), on DVE/ACT ----
                coeff_all = cfp.tile([P, TT // P, E], BF16, tag="coefball")
                for s in range(TT // P):
                    e_sb = smallp.tile([P, E], F32, tag=f"e{s % 2}")
                    nc.scalar.activation(e_sb[:], lg_ps[:, s, :], AF.Exp)
                    m1 = smallp.tile([P, 1], F32, tag="m1")
                    nc.vector.tensor_reduce(m1[:], e_sb[:],
                                            axis=mybir.AxisListType.X,
                                            op=ALU.max)
                    mask1 = smallp.tile([P, E], F32, tag="mask1")
                    nc.vector.tensor_scalar(mask1[:], e_sb[:], m1[:], None,
                                            op0=ALU.is_ge)
                    masked = smallp.tile([P, E], F32, tag="masked")
                    nc.vector.scalar_tensor_tensor(
                        masked[:], in0=mask1[:], scalar=NEG_BIG, in1=e_sb[:],
                        op0=ALU.mult, op1=ALU.add)
                    m2 = smallp.tile([P, 1], F32, tag="m2")
                    nc.vector.tensor_reduce(m2[:], masked[:],
                                            axis=mybir.AxisListType.X,
                                            op=ALU.max)
                    s12 = smallp.tile([P, 1], F32, tag="s12")
                    nc.vector.tensor_tensor(s12[:], m1[:], m2[:], op=ALU.add)
                    rs = smallp.tile([P, 1], F32, tag="rs")
                    nc.vector.reciprocal(rs[:], s12[:])
                    mask2 = smallp.tile([P, E], F32, tag="mask2")
                    nc.vector.tensor_scalar(mask2[:], e_sb[:], m2[:], None,
                                            op0=ALU.is_ge)
                    nc.vector.scalar_tensor_tensor(
                        coeff_all[:, s, :], in0=e_sb[:], scalar=rs[:],
                        in1=mask2[:], op0=ALU.mult, op1=ALU.mult)

                # ---- one PE transpose coeff [128,(4s,16e)] -> [64se,128t] ----
                ct_ps = ps_sm.tile([TT // P * E, P], BF16, tag="lgct")
                nc.tensor.transpose(ct_ps[:], coeff_all[:], ident_sb[:])
                ct_sb = cfp.tile([TT // P * E, P], BF16, tag="ctsb")
                nc.vector.tensor_copy(ct_sb[:], ct_ps[:])

                # ---- expand coeff.T rows to (e,r) rows (scaled /128);
                #      wxa8 = xa_s * cexp quantized to fp8e4 on DVE ----
                wxa_sb = cfp.tile([P, 2, TT], F8, tag="wxa")
                for half in range(2):
                    cx_ps = ps_sm.tile([P, TT], F32, tag="cx")
                    for s in range(TT // P):
                        nc.tensor.matmul(
                            cx_ps[:, s * P:(s + 1) * P],
                            lhsT=ex4_sb[:, s, half, :],
                            rhs=ct_sb[:], start=True, stop=True)
                    cx_sb = cfp.tile([P, TT], F32, tag=f"cxs{half}")
                    nc.vector.tensor_copy(cx_sb[:], cx_ps[:])
                    nc.vector.tensor_tensor(wxa_sb[:, half, :], xa_ps[half][:],
                                            cx_sb[:], op=ALU.mult)
                return xb_sb, wxa_sb

            def drain(ps, oc, t0, tt):
                o_sb = outp.tile([P, TT], BF16)
                # epilogue on ACT: keeps the PSUM drain off DVE, which
                # is busy with the next tile's top-2 chain.  The final
                # chunks drain in halves so the last out-DMA starts as
                # early as possible (shorter kernel tail).
                nseg = 2 if (tt == NT - 1 and oc >= OC - 2) else 1
                sw = TT // nseg
                for sg in range(nseg):
                    nc.scalar.activation(o_sb[:, sg * sw:(sg + 1) * sw],
                                         ps[:, sg * sw:(sg + 1) * sw],
                                         AF.Identity,
                                         bias=bias_sb[:, oc:oc + 1],
                                         scale=1.0)
                    nc.sync.dma_start(
                        out=outT_d[oc * P:(oc + 1) * P,
                                   t0 + sg * sw:t0 + (sg + 1) * sw],
                        in_=o_sb[:, sg * sw:(sg + 1) * sw])

            def back(tt, state):
                # base + delta accumulated per 128-row output chunk; chunks
                # processed in pairs so the bf16->fp8 PE mode switch for the
                # DoubleRow delta matmuls is paid once per pair
                t0 = tt * TT
                xb_sb, wxa_sb = state

                def base_mms(oc):
                    ps = ps_out.tile([P, TT], F32, tag="out")
                    for k in range(KC):
                        nc.tensor.matmul(ps[:], lhsT=wt_sb[:, oc, k, :],
                                         rhs=xb_sb[:, k, :],
                                         start=(k == 0), stop=False)
                    return ps

                def delta_mms(ps, oc):
                    for th in range(2):
                        nc.tensor.matmul(
                            ps[:, th * TH:(th + 1) * TH],
                            lhsT=bf8_sb[:, :, oc * P:(oc + 1) * P],
                            rhs=wxa_sb[:, :, th * TH:(th + 1) * TH],
                            start=False, stop=True,
                            perf_mode=DR, skip_group_check=True)

                # chunks in pairs (PSUM bufs=3 leaves one spare so the next
                # pair never stalls on this pair's drains): the bf16<->fp8
                # switch for the DoubleRow deltas is paid once per pair
                for op in range(OC // 2):
                    ocs = (2 * op, 2 * op + 1)
                    if tt == NT - 1 and op == OC // 2 - 1:
                        # last pair of the run: finish each chunk (delta +
                        # drain) immediately so the final drain+DMA tail
                        # after the last matmul covers just half a chunk
                        for oc in ocs:
                            ps = base_mms(oc)
                            delta_mms(ps, oc)
                            drain(ps, oc, t0, tt)
                    else:
                        pss = [base_mms(oc) for oc in ocs]
                        for half in range(2):
                            delta_mms(pss[half], ocs[half])
                        for half in range(2):
                            drain(pss[half], ocs[half], t0, tt)

            # one-tile-deep software pipeline: front(j+1) fills the PE
            # while back(j) waits on wt / PSUM drains
            states = {0: front(0)}
            for tt in range(NT):
                if tt + 1 < NT:
                    states[tt + 1] = front(tt + 1)
                back(tt, states.pop(tt))

    nc.compile()
    return nc


_CACHE = {}


def _get_nc(n_core: int):
    if n_core not in _CACHE:
        _CACHE[n_core] = _build(n_core)
    return _CACHE[n_core]


def _prep_in_maps(x, W, bias, rw1, rb1, rw2, rb2, A, B, gates):
    x, W, bias, rw1, rb1, rw2, rb2, A, B, gates = (
        np.asarray(v) for v in (x, W, bias, rw1, rb1, rw2, rb2, A, B, gates))
    xf = np.ascontiguousarray(x.reshape(-1, D).astype(np.float32))
    n = xf.shape[0]
    assert n % N_CORES == 0
    n_core = n // N_CORES

    bf16 = ml_dtypes.bfloat16
    f8 = mybir.dt.np(mybir.dt.float8e4)
    xT = np.ascontiguousarray(xf.T)                      # [D, n] f32
    xTb = xT.astype(bf16)
    xT8 = xT.astype(f8)
    # W.T blocked i-major [OC, 128i, KC, 128o]
    wt = np.ascontiguousarray(
        W.astype(np.float32).T.reshape(KC, P, OC, P).transpose(2, 1, 0, 3)
    ).astype(bf16)
    # A.T scaled + k-paired: [P, KCP, 2, ER]
    a32t = (A.astype(np.float32) * SC_A).reshape(ER, D).T  # [D, ER]
    at8 = np.ascontiguousarray(
        a32t.reshape(KCP, 2, P, ER).transpose(2, 0, 1, 3)).astype(f8)
    # B rows scaled + er-paired: [P, 2, D]
    bfl = (B.astype(np.float32) * SC_B).transpose(0, 2, 1).reshape(ER, D)
    bf8 = np.ascontiguousarray(
        bfl.reshape(2, P, D).transpose(1, 0, 2)).astype(f8)
    # router W1.T packed [P, KC, HID]
    rw1t = np.ascontiguousarray(
        rw1.astype(np.float32).T.reshape(KC, P, HID).transpose(1, 0, 2)
    ).astype(bf16)
    rb1c = np.ascontiguousarray(rb1.astype(np.float32).reshape(HID, 1))
    rw2a = np.concatenate(
        [rw2[:E].astype(np.float32).T,
         (rb2[:E].astype(np.float32) + gates.astype(np.float32))[None, :]],
        axis=0)
    rw2a = np.ascontiguousarray(rw2a).astype(bf16)
    biaspp = np.ascontiguousarray(
        bias.astype(np.float32).reshape(OC, P).T)
    # ex4[se, s, h, j] = SC_E iff se's s-block matches and expert(se) owns
    # lora row h*128+j  (se = s*E + e)
    ex4 = np.zeros((4 * E, 4, 2, P), np.float32)
    for s in range(4):
        for e in range(E):
            for j in range(ER):
                ex4[s * E + e, s, j // P, j % P] = (
                    SC_E if j // R == e else 0.0)
    ex4 = ex4.astype(bf16)
    ident = np.eye(P, dtype=np.float32).astype(bf16)

    shared = {"wt": wt, "at8": at8, "bf8": bf8, "rw1t": rw1t, "rb1": rb1c,
              "rw2a": rw2a, "biaspp": biaspp, "ex4": ex4, "ident": ident}
    NT = n_core // TT
    in_maps = []
    for c in range(N_CORES):
        sl = slice(c * n_core, (c + 1) * n_core)
        xc = (xTb[:, sl].reshape(KC, P, NT, TT)
              .transpose(2, 1, 0, 3))
        xc8 = (xT8[:, sl].reshape(KCP, 2, P, NT, TT)
               .transpose(3, 2, 0, 1, 4))
        in_maps.append({"xbf": np.ascontiguousarray(xc),
                        "x8": np.ascontiguousarray(xc8), **shared})
    return in_maps, n_core


def kernel(x, W, bias, rw1, rb1, rw2, rb2, A, B, gates):
    lead = x.shape[:-1]
    in_maps, n_core = _prep_in_maps(x, W, bias, rw1, rb1, rw2, rb2, A, B,
                                    gates)
    n = n_core * N_CORES
    nc = _get_nc(n_core)
    res = None
    for attempt in range(3):
        try:
            res = run_bass_kernel_spmd(nc, in_maps,
                                       core_ids=list(range(N_CORES)))
            break
        except Exception:
            # sporadic NRT_EXEC_UNIT_UNRECOVERABLE on a fresh NEFF; retry
            if attempt == 2:
                raise
            import time as _time

            _time.sleep(10)

    out = np.empty((n, D), np.float32)
    for c in range(N_CORES):
        out[c * n_core:(c + 1) * n_core] = (
            res.results[c]["outT"].astype(np.float32).T)
    return out.reshape(*lead, D)
